# revision 34
# baseline (speedup 1.0000x reference)
"""Trainium2 Bass kernel for nn_DrugGCNncoder (2-layer GCN + max-pool + MLP).

Self-contained: accepts the FULL inputs of reference.setup_inputs(), shards
across 8 NeuronCores internally (dst-node/graph sharding), returns the FULL
[512, 128] output.

v5 design:
  - all-bf16 datapath (x table, gathered tiles, S selection matrices, W1/W2):
    DVE 2x perf mode on S builds, half the gather DMA bytes
  - DSTW=128 windows: each edge's S row spans only 128 dst columns, halving
    the S-build element count (the dominant vector/scalar-engine cost)
  - gathers issued per GROUP of 4 windows (one SWDGE call per (group, table
    chunk)) spread over 4 SWDGE queues, so descriptor generation overlaps
    DMA drain and fixed overhead is paid ~230x instead of ~900x
  - S built on DVE (is_equal*norm) for 60% of tiles and on the scalar engine
    (relu(norm - norm*(iota-dstl)^2)) for 40%, balancing engine load
  - fused h1 epilogue inside the L1 window loop (agg -> W1 matmul -> relu ->
    bf16 h1 block); h1 AllGather in 4 window-aligned pieces overlapping the
    tail of L1 and the head of L2
"""
import sys
for p in ("/opt/trn_rl_repo", "/root/.axon_site/_ro/trn_rl_repo"):
    if p not in sys.path:
        sys.path.insert(0, p)
import numpy as np
import ml_dtypes
import concourse.bass as bass
import concourse.bacc as bacc
import concourse.mybir as mybir
from concourse import tile
from concourse.bass_utils import run_bass_kernel_spmd

BF16 = ml_dtypes.bfloat16

CHUNK = 32768       # x gather table chunk (int16 index range)
DSTW = 128          # window width in dst-node columns (S matrix free dim)
GROUP = 4           # windows per gather group
F1P = 128           # x padded feature count (256B bf16 rows)
F1 = 78
F2S = 384           # h1 stored width (768B bf16 rows)
F2 = 300
FOUT = 128
GMAXI = 1024        # max indices per dma_gather call (>1024 fails on HW)
NPIECE = 4          # h1 AllGather pieces
G_PER_CORE = 64
N_CORES = 8
N_GRAPHS = 512


def _pack_idx16(idx, cap):
    """idx (valid list) -> [128, cap//16] int16, slot j at [j%16, j//16],
    padded with 0 (valid row 0; S has zeros for pad slots; negative skip
    indices are NOT safe on this HW path - NRT_EXEC_UNIT_UNRECOVERABLE),
    replicated 8x along partitions."""
    assert cap % 16 == 0 and len(idx) <= cap
    full = np.zeros(cap, np.int16)
    full[: len(idx)] = idx
    blk = full.reshape(cap // 16, 16).T  # [16, cap/16]
    return np.tile(blk, (8, 1))  # [128, cap/16]


def build_plan(x, edge_index, batch, weights, n_graphs=512, n_cores=8):
    global G_PER_CORE, N_CORES, N_GRAPHS
    N_GRAPHS, N_CORES = n_graphs, n_cores
    G_PER_CORE = n_graphs // n_cores
    N = x.shape[0]
    src = np.concatenate([edge_index[0], np.arange(N)]).astype(np.int64)
    dst = np.concatenate([edge_index[1], np.arange(N)]).astype(np.int64)
    deg = np.bincount(dst, minlength=N).astype(np.float64)
    dis = np.where(deg > 0, 1.0 / np.sqrt(deg), 0.0)
    norm = (dis[src] * dis[dst]).astype(np.float32)

    batch = batch.astype(np.int64)
    g_start = np.searchsorted(batch, np.arange(N_GRAPHS), side="left")
    g_end = np.searchsorted(batch, np.arange(N_GRAPHS), side="right")
    node_start = [int(g_start[c * G_PER_CORE]) for c in range(N_CORES)]
    node_start.append(N)
    nodes_per_core = [node_start[c + 1] - node_start[c] for c in range(N_CORES)]
    # NMAX: multiple of GROUP*DSTW so groups and pieces align
    NMAX = ((max(nodes_per_core) + GROUP * DSTW - 1) // (GROUP * DSTW)) \
        * (GROUP * DSTW)
    n_win1 = NMAX // DSTW

    # ---- h1 pieces (window-aligned) ----------------------------------
    assert n_win1 % NPIECE == 0
    base_w = n_win1 // NPIECE
    piece_wins = [base_w] * NPIECE
    piece_w0 = np.concatenate([[0], np.cumsum(piece_wins)])
    piece_rows = [w * DSTW for w in piece_wins]
    piece_r0 = np.concatenate([[0], np.cumsum(piece_rows)])
    assert n_cores * piece_rows[0] <= 32768

    core_of = np.searchsorted(np.asarray(node_start[1:]), np.arange(N),
                              side="right")
    local_of = np.arange(N) - np.asarray(node_start)[core_of]
    piece_of = np.searchsorted(piece_r0[1:], local_of, side="right")
    prows = np.asarray(piece_rows)[piece_of]
    h1row_of = core_of * prows + (local_of - piece_r0[piece_of])

    per_core_raw = []
    for c in range(N_CORES):
        sel = (dst >= node_start[c]) & (dst < node_start[c + 1])
        s, d, nm = src[sel], dst[sel], norm[sel]
        dl = d - node_start[c]
        order = np.argsort(dl, kind="stable")
        per_core_raw.append((s[order], dl[order], nm[order]))

    def _windows_for_edges(dst_local, base_grid, limits=None):
        out = []
        for i, b in enumerate(base_grid):
            top = b + DSTW if limits is None else min(b + DSTW, limits[i])
            lo = np.searchsorted(dst_local, b, side="left")
            hi = np.searchsorted(dst_local, top, side="left")
            out.append((lo, hi))
        return out

    # window construction: per window, runs keyed by table cell, each run
    # sorted by source row id for DRAM locality.
    def make_windows(core_edges, base_grid, row_ids, cell_of, n_cells,
                     limits=None):
        s_loc, dl, nm = core_edges
        wins = []
        for (lo, hi), b in zip(_windows_for_edges(dl, base_grid, limits),
                               base_grid):
            er, edl, enm = row_ids[lo:hi], dl[lo:hi] - b, nm[lo:hi]
            ec = cell_of[lo:hi]
            runs = []
            for k in range(n_cells):
                m = ec == k
                rr, rd, rn = er[m], edl[m], enm[m]
                o = np.argsort(rr, kind="stable")
                runs.append((rr[o], rd[o], rn[o]))
            wins.append((b, runs))
        return wins

    n_chunks_x = (N + CHUNK - 1) // CHUNK
    l1_cores, l2_cores = [], []
    for c in range(N_CORES):
        s_loc, dl, nm = per_core_raw[c]
        grid1 = np.arange(0, NMAX, DSTW)
        l1_cores.append(make_windows(
            (s_loc, dl, nm), grid1, s_loc % CHUNK, s_loc // CHUNK,
            n_chunks_x))
        base2, lim2 = [], []
        for g in range(c * G_PER_CORE, (c + 1) * G_PER_CORE):
            glo = g_start[g] - node_start[c]
            ghi = g_end[g] - node_start[c]
            for b in range(int(glo), int(ghi), DSTW):
                base2.append(b)
                lim2.append(int(ghi))
        base2 = np.asarray(base2, np.int64)
        l2_cores.append(make_windows(
            (s_loc, dl, nm), base2, h1row_of[s_loc], piece_of[s_loc],
            NPIECE, limits=lim2))

    # ---- capacity normalization across cores ---------------------------
    def normalize(cores_wins, n_cells):
        n_win = max(len(w) for w in cores_wins)
        n_win = ((n_win + GROUP - 1) // GROUP) * GROUP
        for wlist in cores_wins:
            while len(wlist) < n_win:
                wlist.append((0, [(np.array([], np.int64),) * 3] * n_cells))
        caps = np.zeros((n_win, n_cells), np.int64)
        for wlist in cores_wins:
            for w, (b, runs) in enumerate(wlist):
                for k, (ri, rd, rn) in enumerate(runs):
                    caps[w, k] = max(caps[w, k], len(ri))
        caps = ((caps + 127) // 128) * 128
        return n_win, caps

    n_win1_n, caps1 = normalize(l1_cores, n_chunks_x)
    assert n_win1_n == n_win1
    n_win2, caps2 = normalize(l2_cores, NPIECE)

    # per-window tile count and per-group layout
    def layout(caps, n_cells):
        n_win = caps.shape[0]
        n_grp = n_win // GROUP
        Twin = int(caps.sum(axis=1).max()) // 128        # tiles per window
        # group cell slot offsets: cell (g,k) holds [w0|w1|w2|w3] sub-blocks
        # grp_sched entries: (k, cellcap, local_slot, global_c16)
        grp_sched = []
        win_tiles = [[] for _ in range(n_win)]  # per window: gbuf tile idx
        win_cells = [[] for _ in range(n_win)]  # per window: (k, cap, gslot)
        gslot = 0          # global slot (idx16/meta emit offset)
        Tgrp = 0
        for g in range(n_grp):
            lslot = 0      # group-local slot (gbuf offset)
            ent = []
            for k in range(n_cells):
                wcaps = [int(caps[g * GROUP + j, k]) for j in range(GROUP)]
                cellcap = sum(wcaps)
                if cellcap > 0:
                    ent.append((k, cellcap, lslot, gslot // 16))
                off_l, off_g = lslot, gslot
                for j in range(GROUP):
                    w = g * GROUP + j
                    if wcaps[j] > 0:
                        win_cells[w].append((k, wcaps[j], off_g))
                        for t in range(wcaps[j] // 128):
                            win_tiles[w].append(off_l // 128 + t)
                    off_l += wcaps[j]
                    off_g += wcaps[j]
                lslot += cellcap
                gslot += cellcap
            grp_sched.append((ent, lslot))
            Tgrp = max(Tgrp, lslot // 128)
        return n_grp, Twin, Tgrp, grp_sched, win_tiles, win_cells, gslot // 16

    (n_grp1, Twin1, Tgrp1, gsched1, wtiles1, wcells1, n_idx16_1) = \
        layout(caps1, n_chunks_x)
    (n_grp2, Twin2, Tgrp2, gsched2, wtiles2, wcells2, n_idx16_2) = \
        layout(caps2, NPIECE)

    # ---- emit per-core arrays ------------------------------------------
    def emit(cores_wins, caps, n_win, Twin, win_cells, n_idx16, n_cells):
        out = []
        for wlist in cores_wins:
            idx16 = np.zeros((128, n_idx16), np.int16)
            dstl = np.full((n_win, 128, Twin), -1.0, np.float32)
            nrm = np.zeros((n_win, 128, Twin), np.float32)
            for w, (b, runs) in enumerate(wlist):
                tbase = 0
                for (k, cap, slot) in win_cells[w]:
                    ri, rd, rn = runs[k]
                    idx16[:, slot // 16 : (slot + cap) // 16] = \
                        _pack_idx16(ri, cap)
                    n = len(ri)
                    sl = tbase * 128 + np.arange(n)
                    dstl[w, sl % 128, sl // 128] = rd.astype(np.float32)
                    nrm[w, sl % 128, sl // 128] = rn
                    tbase += cap // 128
                assert tbase <= Twin
            meta = np.concatenate([dstl, nrm, -dstl, -nrm], axis=2)
            out.append({"idx16": idx16, "meta": meta})
        return out

    l1_data = emit(l1_cores, caps1, n_win1, Twin1, wcells1, n_idx16_1,
                   n_chunks_x)
    l2_data = emit(l2_cores, caps2, n_win2, Twin2, wcells2, n_idx16_2,
                   NPIECE)

    # ---- pooling masks --------------------------------------------------
    n_win2_pad16 = ((n_win2 + 15) // 16) * 16
    pool_masks = []
    for c in range(N_CORES):
        m = np.full((G_PER_CORE, n_win2_pad16), np.float32(-3.0e38),
                    np.float32)
        wlist = l2_cores[c]
        glo = g_start[c * G_PER_CORE : (c + 1) * G_PER_CORE] - node_start[c]
        ghi = g_end[c * G_PER_CORE : (c + 1) * G_PER_CORE] - node_start[c]
        for w, (b, runs) in enumerate(wlist):
            g = int(np.searchsorted(ghi, b, side="right"))
            if g < G_PER_CORE and glo[g] <= b < ghi[g]:
                m[g, w] = 0.0
        pool_masks.append(m)
    for c in range(N_CORES):
        seen = set()
        wlist = l2_cores[c]
        for w, (b, runs) in enumerate(wlist):
            total = sum(len(r[0]) for r in runs)
            key = int(b)
            if total == 0 and key in seen:
                pool_masks[c][:, w] = -3.0e38
            seen.add(key)

    # ---- packed weights (shared across cores) --------------------------
    W1, b1, W2, b2, W3, b3, W4, b4 = (
        weights["W1"], weights["b1"], weights["W2"], weights["b2"],
        weights["W3"], weights["b3"], weights["W4"], weights["b4"],
    )
    W1aug = np.zeros((80, F2S), np.float32)
    W1aug[:F1, :F2] = W1
    W1aug[F1, :F2] = b1          # ones-col slot 78
    W2aug = np.zeros((304, 384), np.float32)
    W2aug[:F2, :F2] = W2
    W2aug[F2, :F2] = b2
    W3aug = np.zeros((304, 1024), np.float32)
    W3aug[:F2, :] = W3
    W3aug[F2, :] = b3
    W4aug = np.zeros((1152, FOUT), np.float32)
    W4aug[:1024, :] = W4
    W4aug[1024, :] = b4

    x_pad = np.zeros((N, F1P), BF16)
    x_pad[:, :F1] = x.astype(BF16)
    x_pad[:, F1] = BF16(1.0)

    cfg = dict(
        G_PER_CORE=G_PER_CORE, n_cores=N_CORES,
        N=N, NMAX=NMAX, n_win1=n_win1, n_win2=n_win2,
        n_grp1=n_grp1, n_grp2=n_grp2,
        Twin1=Twin1, Twin2=Twin2, Tgrp1=Tgrp1, Tgrp2=Tgrp2,
        gsched1=gsched1, gsched2=gsched2,
        wtiles1=wtiles1, wtiles2=wtiles2,
        n_chunks_x=n_chunks_x,
        n_idx16_1=n_idx16_1, n_idx16_2=n_idx16_2,
        n_win2_pad16=n_win2_pad16,
        piece_wins=piece_wins, piece_rows=piece_rows,
        piece_w0=[int(v) for v in piece_w0],
        piece_r0=[int(v) for v in piece_r0],
    )
    onesmat = np.zeros((128, 256), np.float32)
    onesmat[0, :] = 1.0
    shared = dict(
        W1aug=W1aug.astype(BF16), W2aug=W2aug.astype(BF16),
        W3aug=W3aug, W4aug=W4aug, x_pad=x_pad,
        onesmat=onesmat, onesb16=np.ones((1, 256), BF16),
    )
    per_core = []
    for c in range(N_CORES):
        per_core.append(dict(
            idx16_1=l1_data[c]["idx16"], meta1=l1_data[c]["meta"],
            idx16_2=l2_data[c]["idx16"], meta2=l2_data[c]["meta"],
            pool_mask_bcast=np.tile(pool_masks[c][:, None, :], (1, 128, 1)),
            node_start=node_start[c], n_nodes=nodes_per_core[c],
        ))
    return cfg, per_core, shared


FP32 = mybir.dt.float32
FP32R = mybir.dt.float32r
BF16D = mybir.dt.bfloat16
I16 = mybir.dt.int16
AF = mybir.ActivationFunctionType
ALU = mybir.AluOpType


def build_kernel(cfg, n_cores=8):
    G = cfg["G_PER_CORE"]
    N, NMAX = cfg["N"], cfg["NMAX"]
    n_win1, n_win2 = cfg["n_win1"], cfg["n_win2"]
    n_grp1, n_grp2 = cfg["n_grp1"], cfg["n_grp2"]
    Twin1, Twin2 = cfg["Twin1"], cfg["Twin2"]
    Tgrp1, Tgrp2 = cfg["Tgrp1"], cfg["Tgrp2"]
    gsched1, gsched2 = cfg["gsched1"], cfg["gsched2"]
    wtiles1, wtiles2 = cfg["wtiles1"], cfg["wtiles2"]
    n_win2p = cfg["n_win2_pad16"]
    piece_rows = cfg["piece_rows"]
    piece_w0 = cfg["piece_w0"]
    piece_r0 = cfg["piece_r0"]

    nc = bacc.Bacc("TRN2", target_bir_lowering=False, debug=False,
                   num_devices=n_cores, num_swdge_queues=4)

    # ---- I/O ----
    x_pad = nc.dram_tensor("x_pad", [N, F1P], BF16D, kind="ExternalInput")
    idx1 = nc.dram_tensor("idx1", [128, cfg["n_idx16_1"]], I16,
                          kind="ExternalInput")
    idx2 = nc.dram_tensor("idx2", [128, cfg["n_idx16_2"]], I16,
                          kind="ExternalInput")
    meta1 = nc.dram_tensor("meta1", [n_win1, 128, 4 * Twin1], FP32,
                           kind="ExternalInput")
    meta2 = nc.dram_tensor("meta2", [n_win2, 128, 4 * Twin2], FP32,
                           kind="ExternalInput")
    pmask = nc.dram_tensor("pmask", [G, 128, n_win2p], FP32,
                           kind="ExternalInput")
    w1aug = nc.dram_tensor("w1aug", [80, F2S], BF16D, kind="ExternalInput")
    w2aug = nc.dram_tensor("w2aug", [304, 384], BF16D, kind="ExternalInput")
    w3aug = nc.dram_tensor("w3aug", [304, 1024], FP32, kind="ExternalInput")
    w4aug = nc.dram_tensor("w4aug", [1152, 128], FP32, kind="ExternalInput")
    onesmat = nc.dram_tensor("onesmat", [128, 256], FP32,
                             kind="ExternalInput")
    onesb16 = nc.dram_tensor("onesb16", [1, 256], BF16D,
                             kind="ExternalInput")
    z_out = nc.dram_tensor("z", [G, 128], FP32, kind="ExternalOutput")

    with tile.TileContext(nc) as tc, \
         tc.tile_pool(name="dram", bufs=1, space="DRAM") as drp, \
         tc.tile_pool(name="consts", bufs=1) as consts:
        # ---- persistent DRAM intermediates ----
        h1_me = [drp.tile([piece_rows[q], F2S], BF16D, name=f"h1me{q}")
                 for q in range(NPIECE)]
        h1_full = [drp.tile([n_cores * piece_rows[q], F2S], BF16D,
                            addr_space="Shared", name=f"h1full{q}")
                   for q in range(NPIECE)]

        iota_i32 = consts.tile([128, DSTW], mybir.dt.int32)
        nc.gpsimd.iota(iota_i32[:], [[1, DSTW]], base=0, channel_multiplier=0)
        iota_sb = consts.tile([128, DSTW], BF16D)
        nc.vector.tensor_copy(iota_sb[:], iota_i32[:])

        w1_sb = consts.tile([80, F2S], BF16D)
        nc.sync.dma_start(w1_sb[:], w1aug[:])
        w2_sb = []
        for k in range(3):
            rows = [128, 128, 48][k]
            t = consts.tile([rows, 384], BF16D, name=f"w2_sb{k}")
            nc.sync.dma_start(t[:], w2aug[k * 128 : k * 128 + rows, :])
            w2_sb.append(t)
        w2b_sb = consts.tile([1, 384], BF16D)
        nc.sync.dma_start(w2b_sb[:], w2aug[300:301, :])
        w3_sb = []
        for k in range(3):
            rows = [128, 128, 48][k]
            t = consts.tile([rows, 1024], FP32R, name=f"w3_sb{k}")
            nc.sync.dma_start(t[:], w3aug[k * 128 : k * 128 + rows, :].bitcast(FP32R))
            w3_sb.append(t)
        w3b_sb = consts.tile([1, 1024], FP32R)
        nc.sync.dma_start(w3b_sb[:], w3aug[300:301, :].bitcast(FP32R))
        w4_sb = []
        for k in range(9):
            t = consts.tile([128, 128], FP32R, name=f"w4_sb{k}")
            nc.sync.dma_start(t[:], w4aug[k * 128 : (k + 1) * 128, :].bitcast(FP32R))
            w4_sb.append(t)
        ones128_sb = consts.tile([1, DSTW], BF16D)
        nc.sync.dma_start(ones128_sb[:], onesb16[0:1, 0:DSTW])
        ones_sb = consts.tile([128, G], FP32R)
        nc.sync.dma_start(ones_sb[:], onesmat[:, 0:G].bitcast(FP32R))

        pooled_win = [consts.tile([128, n_win2p], FP32, name=f"pw{m}")
                      for m in range(3)]
        for m in range(3):
            nc.vector.memset(pooled_win[m][:], -3.0e38)

        # =============== group gather ===============
        qctr = [0]

        def gather_group(pools, g, gsched, idx_hbm, tables, TG, F, tag):
            gpool, ipool = pools
            ent, tot = gsched[g]
            if not ent:
                return None
            gbuf = gpool.tile([128, TG, F], BF16D, tag="gbuf",
                              name=f"gbuf_{tag}_{g}",
                              padded_shape=[128, TG, F])
            c16_0 = ent[0][3]
            c16_n = ent[-1][3] + ent[-1][1] // 16
            itile = ipool.tile([128, c16_n - c16_0], I16, tag="idx",
                               name=f"idx_{tag}_{g}")
            nc.sync.dma_start(itile[:], idx_hbm[:, c16_0:c16_n])
            for (k, cap, slot, c16) in ent:
                src = tables[k]
                for off in range(0, cap, GMAXI):
                    sub = min(GMAXI, cap - off)
                    so = slot + off
                    co = c16 - c16_0 + off // 16
                    nc.gpsimd.dma_gather(
                        gbuf[:, so // 128 : (so + sub) // 128, :],
                        src,
                        itile[:, co : co + sub // 16],
                        sub, sub, F,
                        queue_num=qctr[0] % 4,
                    )
                    qctr[0] += 1
            return gbuf

        def build_S(spool, upool, meta, T, w, t, tidx, tag):
            """S tile for window w, local tile t (gbuf tile tidx)."""
            S = spool.tile([128, DSTW], BF16D, tag="S",
                           name=f"S{tag}_{w}_{t}")
            if (w + t) % 5 < 3:
                nc.vector.tensor_scalar(
                    S[:], iota_sb[:], meta[:, t : t + 1],
                    meta[:, T + t : T + t + 1], ALU.is_equal, ALU.mult)
            else:
                u = upool.tile([128, DSTW], BF16D, tag="u",
                               name=f"u{tag}_{w}_{t}")
                nc.scalar.activation(
                    u[:], iota_sb[:], AF.Square,
                    bias=meta[:, 2 * T + t : 2 * T + t + 1])
                nc.scalar.activation(
                    S[:], u[:], AF.Relu,
                    bias=meta[:, T + t : T + t + 1],
                    scale=meta[:, 3 * T + t : 3 * T + t + 1])
            return S

        # =============== Phase 1: L1 aggregation + fused h1 ===============
        x_tables = []
        for k in range(cfg["n_chunks_x"]):
            lo = k * CHUNK
            hi = min(lo + CHUNK, N)
            x_tables.append(x_pad[lo:hi, :])
        with tc.tile_pool(name="gp1", bufs=3) as gpool, \
             tc.tile_pool(name="ip1", bufs=4) as ipool, \
             tc.tile_pool(name="mp1", bufs=4) as mpool, \
             tc.tile_pool(name="sp1", bufs=8) as spool, \
             tc.tile_pool(name="up1", bufs=4) as upool, \
             tc.tile_pool(name="ps_agg1", bufs=4, space="PSUM") as ps_agg, \
             tc.tile_pool(name="ps_h1", bufs=2, space="PSUM") as ps_h1, \
             tc.tile_pool(name="sb_ep1", bufs=3) as sb_ep:
            for g in range(n_grp1):
                gbuf = gather_group((gpool, ipool), g, gsched1, idx1,
                                    x_tables, Tgrp1, F1P, "l1")
                if gbuf is None:
                    continue
                for j in range(GROUP):
                    w = g * GROUP + j
                    tl = wtiles1[w]
                    nt = len(tl)
                    if nt == 0:
                        continue
                    meta = mpool.tile([128, 4 * Twin1], FP32, tag="meta",
                                      name=f"m1_{w}")
                    nc.sync.dma_start(meta[:], meta1[w])
                    agg = ps_agg.tile([128, DSTW], FP32, tag="agg1",
                                      name=f"agg1_{w}")
                    for t in range(nt):
                        S = build_S(spool, upool, meta, Twin1, w, t, tl[t],
                                    "1")
                        nc.tensor.matmul(agg[:], gbuf[:, tl[t], :], S[:],
                                         start=(t == 0), stop=(t == nt - 1))
                    # epilogue: agg [80,128] -> h1 block [128,384] -> h1_me
                    agg_sb = sb_ep.tile([80, DSTW], BF16D, tag="agg_sb",
                                        name=f"as1_{w}")
                    nc.scalar.activation(agg_sb[:], agg[0:80, :], AF.Copy)
                    q = int(np.searchsorted(np.asarray(piece_w0[1:]), w,
                                            side="right"))
                    hp = ps_h1.tile([128, F2S], FP32, tag="h1p",
                                    name=f"h1p_{w}")
                    nc.tensor.matmul(hp[:], agg_sb[:], w1_sb[:],
                                     start=True, stop=True)
                    h1b = sb_ep.tile([128, F2S], BF16D, tag="h1b",
                                     name=f"h1b_{w}")
                    nc.scalar.activation(h1b[:], hp[:], AF.Relu)
                    r0 = w * DSTW - piece_r0[q]
                    nc.sync.dma_start(h1_me[q][r0 : r0 + 128, :], h1b[:])

        # =============== Phase 2: piecewise AllGather ===============
        for q in range(NPIECE):
            nc.gpsimd.collective_compute(
                "AllGather", ALU.bypass,
                replica_groups=[list(range(n_cores))],
                ins=[h1_me[q].opt()],
                outs=[h1_full[q].opt()],
            )

        # =============== Phase 3: L2 aggregation + W2 + window pooling =====
        FCH = [(0, 128), (128, 256), (256, 384)]
        KCH = [(0, 128), (128, 256), (256, 300)]
        h1_tables = [h1_full[q][:, :] for q in range(NPIECE)]
        with tc.tile_pool(name="gp2", bufs=2) as gpool, \
             tc.tile_pool(name="ip2", bufs=4) as ipool, \
             tc.tile_pool(name="mp2", bufs=4) as mpool, \
             tc.tile_pool(name="sp2", bufs=8) as spool, \
             tc.tile_pool(name="up2", bufs=4) as upool, \
             tc.tile_pool(name="ps_agg2", bufs=2, space="PSUM") as ps_agg2, \
             tc.tile_pool(name="ps_h2", bufs=2, space="PSUM") as ps_h2, \
             tc.tile_pool(name="sb_ep2", bufs=2) as sb_ep2:
            for g in range(n_grp2):
                gbuf = gather_group((gpool, ipool), g, gsched2, idx2,
                                    h1_tables, Tgrp2, F2S, "l2")
                if gbuf is None:
                    continue
                for j in range(GROUP):
                    w = g * GROUP + j
                    tl = wtiles2[w]
                    nt = len(tl)
                    if nt == 0:
                        continue
                    meta = mpool.tile([128, 4 * Twin2], FP32, tag="meta",
                                      name=f"m2_{w}")
                    nc.sync.dma_start(meta[:], meta2[w])
                    aggs = []
                    for fi, (f0, f1) in enumerate(FCH):
                        aggs.append(ps_agg2.tile([f1 - f0, DSTW], FP32,
                                                 tag=f"agg2_{fi}",
                                                 name=f"agg2_{w}_{fi}"))
                    for t in range(nt):
                        S = build_S(spool, upool, meta, Twin2, w, t, tl[t],
                                    "2")
                        for fi, (f0, f1) in enumerate(FCH):
                            nc.tensor.matmul(aggs[fi][:],
                                             gbuf[:, tl[t], f0:f1], S[:],
                                             start=(t == 0),
                                             stop=(t == nt - 1))
                    a_sb = []
                    for fi, (f0, f1) in enumerate(FCH):
                        t_ = sb_ep2.tile([f1 - f0, DSTW], BF16D,
                                         tag=f"a2sb_{fi}",
                                         name=f"a2sb_{w}_{fi}")
                        nc.scalar.activation(t_[:], aggs[fi][:], AF.Copy)
                        a_sb.append(t_)
                    for m, (m0, m1) in enumerate([(0, 128), (128, 256),
                                                  (256, 384)]):
                        hp = ps_h2.tile([m1 - m0, DSTW], FP32, tag="h2p",
                                        name=f"h2p_{w}_{m}")
                        for ki, (k0, k1) in enumerate(KCH):
                            nc.tensor.matmul(
                                hp[:], w2_sb[ki][0 : k1 - k0, m0:m1],
                                a_sb[ki][0 : k1 - k0, :],
                                start=(ki == 0), stop=False)
                        nc.tensor.matmul(hp[:], w2b_sb[:, m0:m1],
                                         ones128_sb[:], start=False,
                                         stop=True)
                        h2sb = sb_ep2.tile([m1 - m0, DSTW], FP32, tag="h2sb",
                                           name=f"h2sb_{w}_{m}")
                        nc.scalar.activation(h2sb[:], hp[:], AF.Relu)
                        nc.vector.tensor_reduce(
                            pooled_win[m][0 : m1 - m0, w : w + 1], h2sb[:],
                            axis=mybir.AxisListType.X, op=ALU.max)

        # =============== Phase 4: pool combine + MLP ===============
        with tc.tile_pool(name="pm", bufs=3) as pmp, \
             tc.tile_pool(name="pool5", bufs=2) as p5, \
             tc.tile_pool(name="ps_z", bufs=2, space="PSUM") as psz, \
             tc.tile_pool(name="zsb", bufs=2) as zsb:
            pooledT = [p5.tile([128, G], FP32, tag=f"pT{m}", bufs=1,
                               name=f"pooledT{m}") for m in range(3)]
            for g in range(G):
                msk = pmp.tile([128, n_win2p], FP32, tag="msk", name=f"msk_{g}")
                nc.sync.dma_start(msk[:], pmask[g])
                for m in range(3):
                    tmp = pmp.tile([128, n_win2p], FP32, tag="tmp",
                                   name=f"tmp_{g}_{m}")
                    nc.vector.tensor_tensor(tmp[:], pooled_win[m][:], msk[:],
                                            ALU.add)
                    nc.vector.tensor_reduce(
                        pooledT[m][:, g : g + 1], tmp[:],
                        axis=mybir.AxisListType.X, op=ALU.max)
            pooledTr = [p5.tile([128, G], FP32R, tag=f"pTr{m}", bufs=1,
                                name=f"pooledTr{m}") for m in range(3)]
            for m in range(3):
                nc.scalar.activation(pooledTr[m][:], pooledT[m][:], AF.Relu)
            z1t = []
            for mi in range(8):
                zp = psz.tile([128, G], FP32, tag="z1p", name=f"z1p_{mi}")
                for ki, (k0, k1) in enumerate(KCH):
                    nc.tensor.matmul(
                        zp[:], w3_sb[ki][0 : k1 - k0, mi * 128 : (mi + 1) * 128],
                        pooledTr[ki][0 : k1 - k0, :],
                        start=(ki == 0), stop=False)
                nc.tensor.matmul(zp[:], w3b_sb[:, mi * 128 : (mi + 1) * 128],
                                 ones_sb[0:1, :], start=False, stop=True)
                zt = zsb.tile([128, G], FP32R, tag=f"z1t{mi}", bufs=1,
                              name=f"z1t_{mi}")
                nc.scalar.activation(zt[:], zp[:], AF.Relu)
                z1t.append(zt)
            zp2 = psz.tile([G, 128], FP32, tag="z2p", name="z2p")
            for ki in range(9):
                lhsT = z1t[ki][:] if ki < 8 else ones_sb[:]
                nc.tensor.matmul(zp2[:], lhsT, w4_sb[ki][:],
                                 start=(ki == 0), stop=(ki == 8))
            zfin = zsb.tile([G, 128], FP32, tag="zfin", name="zfin")
            nc.scalar.activation(zfin[:], zp2[:], AF.Relu)
            nc.sync.dma_start(z_out[:], zfin[:])

    nc.compile()
    nc.generate_event_semaphores()
    return nc


# ======================= public entry point =======================
_NC_CACHE = {}


def _make_in_maps(cfg, per_core, shared):
    base = dict(
        x_pad=shared["x_pad"], w1aug=shared["W1aug"], w2aug=shared["W2aug"],
        w3aug=shared["W3aug"], w4aug=shared["W4aug"],
        onesmat=shared["onesmat"], onesb16=shared["onesb16"],
    )
    maps = []
    for pc in per_core:
        m = dict(base)
        m["idx1"] = pc["idx16_1"]
        m["idx2"] = pc["idx16_2"]
        m["meta1"] = pc["meta1"]
        m["meta2"] = pc["meta2"]
        m["pmask"] = pc["pool_mask_bcast"]
        maps.append(m)
    return maps


def kernel(x, edge_index, batch, W1, b1, W2, b2, W3, b3, W4, b4,
           trace=False):
    weights = dict(W1=np.asarray(W1, np.float32), b1=np.asarray(b1, np.float32),
                   W2=np.asarray(W2, np.float32), b2=np.asarray(b2, np.float32),
                   W3=np.asarray(W3, np.float32), b3=np.asarray(b3, np.float32),
                   W4=np.asarray(W4, np.float32), b4=np.asarray(b4, np.float32))
    n_graphs = 512
    n_cores = 8
    cfg, per_core, shared = build_plan(
        np.asarray(x, np.float32), np.asarray(edge_index), np.asarray(batch),
        weights, n_graphs=n_graphs, n_cores=n_cores)
    key = (cfg["N"], cfg["NMAX"], cfg["n_win1"], cfg["n_win2"], cfg["Twin1"],
           cfg["Twin2"], cfg["n_idx16_1"], cfg["n_idx16_2"],
           cfg["n_win2_pad16"])
    if key not in _NC_CACHE:
        _NC_CACHE[key] = build_kernel(cfg, n_cores=n_cores)
    nc = _NC_CACHE[key]
    maps = _make_in_maps(cfg, per_core, shared)
    res = run_bass_kernel_spmd(nc, maps, core_ids=list(range(n_cores)),
                               trace=trace)
    z = np.concatenate([res.results[c]["z"] for c in range(n_cores)], axis=0)
    if trace:
        kernel.last_results = res
    return z.astype(np.float32)


# revision 37
# speedup vs baseline: 1.3139x; 1.3139x over previous
"""Trainium2 Bass kernel for nn_DrugGCNncoder (2-layer GCN + max-pool + MLP).

Self-contained: accepts the FULL inputs of reference.setup_inputs(), shards
across 8 NeuronCores internally (dst-node/graph sharding), returns the FULL
[512, 128] output.

v2 design (vs baseline):
  - all-bf16 datapath: x table, gathered tiles, S selection matrices, W1/W2
    (unlocks the DVE 2x/4x perf modes for the S-build tensor_scalar ops and
    halves gather DMA bytes)
  - fused h1 epilogue inside the L1 window loop (agg -> W1 matmul -> relu ->
    bf16 h1 block) - no aggx DRAM roundtrip, no transposes, no redundant
    phase-3 recompute on every core
  - h1 AllGather in 4 pieces (window-aligned) so the collective overlaps the
    tail of L1 and the head of L2
  - pad gather indices are -1 (skipped by the SWDGE ucode) instead of row 0,
    and gather cells are sorted by source row for HBM locality
  - bigger gather batches (4096) to amortize SWDGE fixed overhead
"""
import sys
for p in ("/opt/trn_rl_repo", "/root/.axon_site/_ro/trn_rl_repo"):
    if p not in sys.path:
        sys.path.insert(0, p)
import numpy as np
import ml_dtypes
import concourse.bass as bass
import concourse.bacc as bacc
import concourse.mybir as mybir
from concourse import tile
from concourse.bass_utils import run_bass_kernel_spmd

BF16 = ml_dtypes.bfloat16

CHUNK = 32768       # x gather table chunk (int16 index range)
DSTW = 256          # window width in dst-node columns (S matrix free dim)
F1P = 128           # x padded feature count (256B bf16 rows)
F1 = 78
F2S = 384           # h1 stored width (768B bf16 rows)
F2 = 300
FOUT = 128
GMAXI = 1024        # max indices per dma_gather call
NPIECE = 4          # h1 AllGather pieces
G_PER_CORE = 64
N_CORES = 8
N_GRAPHS = 512


def _pack_idx16(idx, cap):
    """idx (valid list) -> [128, cap//16] int16, slot j at [j%16, j//16],
    padded with 0 (valid row 0; S has zeros for pad slots; negative skip
    indices are NOT safe on this HW path - NRT_EXEC_UNIT_UNRECOVERABLE),
    replicated 8x along partitions."""
    assert cap % 16 == 0 and len(idx) <= cap
    full = np.zeros(cap, np.int16)
    full[: len(idx)] = idx
    blk = full.reshape(cap // 16, 16).T  # [16, cap/16]
    return np.tile(blk, (8, 1))  # [128, cap/16]


def build_plan(x, edge_index, batch, weights, n_graphs=512, n_cores=8):
    global G_PER_CORE, N_CORES, N_GRAPHS
    N_GRAPHS, N_CORES = n_graphs, n_cores
    G_PER_CORE = n_graphs // n_cores
    N = x.shape[0]
    src = np.concatenate([edge_index[0], np.arange(N)]).astype(np.int64)
    dst = np.concatenate([edge_index[1], np.arange(N)]).astype(np.int64)
    deg = np.bincount(dst, minlength=N).astype(np.float64)
    dis = np.where(deg > 0, 1.0 / np.sqrt(deg), 0.0)
    norm = (dis[src] * dis[dst]).astype(np.float32)

    batch = batch.astype(np.int64)
    g_start = np.searchsorted(batch, np.arange(N_GRAPHS), side="left")
    g_end = np.searchsorted(batch, np.arange(N_GRAPHS), side="right")
    node_start = [int(g_start[c * G_PER_CORE]) for c in range(N_CORES)]
    node_start.append(N)
    nodes_per_core = [node_start[c + 1] - node_start[c] for c in range(N_CORES)]
    NMAX = ((max(nodes_per_core) + DSTW - 1) // DSTW) * DSTW
    n_win1 = NMAX // DSTW

    # ---- h1 pieces (window-aligned) ----------------------------------
    base_w = n_win1 // NPIECE
    extra = n_win1 - base_w * NPIECE
    piece_wins = [base_w + (1 if q < extra else 0) for q in range(NPIECE)]
    piece_w0 = np.concatenate([[0], np.cumsum(piece_wins)])  # window offsets
    piece_rows = [w * DSTW for w in piece_wins]              # local rows/piece
    piece_r0 = np.concatenate([[0], np.cumsum(piece_rows)])

    # global padded id: node n in core c at local i
    core_of = np.searchsorted(np.asarray(node_start[1:]), np.arange(N),
                              side="right")
    local_of = np.arange(N) - np.asarray(node_start)[core_of]
    # piece of each node + row inside that piece's gathered table
    piece_of = np.searchsorted(piece_r0[1:], local_of, side="right")
    prows = np.asarray(piece_rows)[piece_of]
    h1row_of = core_of * prows + (local_of - piece_r0[piece_of])

    per_core_raw = []
    for c in range(N_CORES):
        sel = (dst >= node_start[c]) & (dst < node_start[c + 1])
        s, d, nm = src[sel], dst[sel], norm[sel]
        dl = d - node_start[c]
        order = np.argsort(dl, kind="stable")
        per_core_raw.append((s[order], dl[order], nm[order]))

    def _windows_for_edges(dst_local, base_grid, limits=None):
        out = []
        for i, b in enumerate(base_grid):
            top = b + DSTW if limits is None else min(b + DSTW, limits[i])
            lo = np.searchsorted(dst_local, b, side="left")
            hi = np.searchsorted(dst_local, top, side="left")
            out.append((lo, hi))
        return out

    # window construction: runs keyed by table cell (chunk/piece), each run
    # sorted by source row id for DRAM locality.
    def make_windows(core_edges, base_grid, row_ids, cell_of, n_cells,
                     limits=None):
        s_loc, dl, nm = core_edges
        wins = []
        for (lo, hi), b in zip(_windows_for_edges(dl, base_grid, limits),
                               base_grid):
            er, edl, enm = row_ids[lo:hi], dl[lo:hi] - b, nm[lo:hi]
            ec = cell_of[lo:hi]
            runs = []
            for k in range(n_cells):
                m = ec == k
                rr, rd, rn = er[m], edl[m], enm[m]
                o = np.argsort(rr, kind="stable")
                runs.append((rr[o], rd[o], rn[o]))
            wins.append((b, runs))
        return wins

    n_chunks_x = (N + CHUNK - 1) // CHUNK
    l1_cores, l2_cores = [], []
    for c in range(N_CORES):
        s_loc, dl, nm = per_core_raw[c]
        grid1 = np.arange(0, NMAX, DSTW)
        l1_cores.append(make_windows(
            (s_loc, dl, nm), grid1, s_loc % CHUNK, s_loc // CHUNK,
            n_chunks_x))
        base2, lim2 = [], []
        for g in range(c * G_PER_CORE, (c + 1) * G_PER_CORE):
            glo = g_start[g] - node_start[c]
            ghi = g_end[g] - node_start[c]
            for b in range(int(glo), int(ghi), DSTW):
                base2.append(b)
                lim2.append(int(ghi))
        base2 = np.asarray(base2, np.int64)
        l2_cores.append(make_windows(
            (s_loc, dl, nm), base2, h1row_of[s_loc], piece_of[s_loc],
            NPIECE, limits=lim2))

    # ---- capacity normalization across cores ---------------------------
    def normalize(cores_wins, n_cells):
        n_win = max(len(w) for w in cores_wins)
        for wlist in cores_wins:
            while len(wlist) < n_win:
                wlist.append((0, [(np.array([], np.int64),) * 3] * n_cells))
        caps = np.zeros((n_win, n_cells), np.int64)
        for wlist in cores_wins:
            for w, (b, runs) in enumerate(wlist):
                for k, (ri, rd, rn) in enumerate(runs):
                    caps[w, k] = max(caps[w, k], len(ri))
        caps = ((caps + 127) // 128) * 128
        t_tiles = int(caps.sum(axis=1).max()) // 128
        return n_win, caps, t_tiles

    n_win1_n, caps1, T1 = normalize(l1_cores, n_chunks_x)
    assert n_win1_n == n_win1
    n_win2, caps2, T2 = normalize(l2_cores, NPIECE)

    # ---- emit per-core arrays ------------------------------------------
    def emit(cores_wins, caps, n_win, t_tiles, n_cells):
        n_idx16 = int(caps.sum()) // 16
        out = []
        for wlist in cores_wins:
            idx16 = np.zeros((128, n_idx16), np.int16)
            dstl = np.full((n_win, 128, t_tiles), -1.0, np.float32)
            nrm = np.zeros((n_win, 128, t_tiles), np.float32)
            col16 = 0
            for w, (b, runs) in enumerate(wlist):
                slot = 0
                for k in range(n_cells):
                    cap = int(caps[w, k])
                    ri, rd, rn = runs[k]
                    idx16[:, col16 : col16 + cap // 16] = _pack_idx16(ri, cap)
                    n = len(ri)
                    sl = slot + np.arange(n)
                    dstl[w, sl % 128, sl // 128] = rd.astype(np.float32)
                    nrm[w, sl % 128, sl // 128] = rn
                    slot += cap
                    col16 += cap // 16
                assert slot <= t_tiles * 128
            # planes: dstl, norm, -dstl (ACT square bias), -norm (ACT scale)
            meta = np.concatenate([dstl, nrm, -dstl, -nrm], axis=2)
            out.append({"idx16": idx16, "meta": meta})
        return out

    l1_data = emit(l1_cores, caps1, n_win1, T1, n_chunks_x)
    l2_data = emit(l2_cores, caps2, n_win2, T2, NPIECE)

    def sched(caps):
        rows = []
        col16 = 0
        for w in range(caps.shape[0]):
            slot = 0
            ent = []
            for k in range(caps.shape[1]):
                cap = int(caps[w, k])
                if cap > 0:
                    ent.append((k, cap, slot, col16))
                slot += cap
                col16 += cap // 16
            rows.append((ent, slot))
        return rows

    # ---- pooling masks --------------------------------------------------
    n_win2_pad16 = ((n_win2 + 15) // 16) * 16
    pool_masks = []
    for c in range(N_CORES):
        m = np.full((G_PER_CORE, n_win2_pad16), np.float32(-3.0e38),
                    np.float32)
        wlist = l2_cores[c]
        glo = g_start[c * G_PER_CORE : (c + 1) * G_PER_CORE] - node_start[c]
        ghi = g_end[c * G_PER_CORE : (c + 1) * G_PER_CORE] - node_start[c]
        for w, (b, runs) in enumerate(wlist):
            g = int(np.searchsorted(ghi, b, side="right"))
            if g < G_PER_CORE and glo[g] <= b < ghi[g]:
                m[g, w] = 0.0
        pool_masks.append(m)
    # pad windows alias base 0; only the first (base,graph) keeps its mask
    for c in range(N_CORES):
        seen = set()
        wlist = l2_cores[c]
        for w, (b, runs) in enumerate(wlist):
            total = sum(len(r[0]) for r in runs)
            key = int(b)
            if total == 0 and key in seen:
                pool_masks[c][:, w] = -3.0e38
            seen.add(key)

    # ---- packed weights (shared across cores) --------------------------
    W1, b1, W2, b2, W3, b3, W4, b4 = (
        weights["W1"], weights["b1"], weights["W2"], weights["b2"],
        weights["W3"], weights["b3"], weights["W4"], weights["b4"],
    )
    W1aug = np.zeros((80, F2S), np.float32)
    W1aug[:F1, :F2] = W1
    W1aug[F1, :F2] = b1          # ones-col slot 78
    W2aug = np.zeros((304, 384), np.float32)
    W2aug[:F2, :F2] = W2
    W2aug[F2, :F2] = b2
    W3aug = np.zeros((304, 1024), np.float32)
    W3aug[:F2, :] = W3
    W3aug[F2, :] = b3
    W4aug = np.zeros((1152, FOUT), np.float32)
    W4aug[:1024, :] = W4
    W4aug[1024, :] = b4

    x_pad = np.zeros((N, F1P), BF16)
    x_pad[:, :F1] = x.astype(BF16)
    x_pad[:, F1] = BF16(1.0)

    cfg = dict(
        G_PER_CORE=G_PER_CORE, n_cores=N_CORES,
        N=N, NMAX=NMAX, n_win1=n_win1, n_win2=n_win2,
        T1=T1, T2=T2, n_chunks_x=n_chunks_x,
        sched1=sched(caps1), sched2=sched(caps2),
        n_idx16_1=int(caps1.sum()) // 16, n_idx16_2=int(caps2.sum()) // 16,
        n_win2_pad16=n_win2_pad16,
        piece_wins=piece_wins, piece_rows=piece_rows,
        piece_w0=[int(v) for v in piece_w0],
        piece_r0=[int(v) for v in piece_r0],
    )
    onesmat = np.zeros((128, DSTW), np.float32)
    onesmat[0, :] = 1.0
    shared = dict(
        W1aug=W1aug.astype(BF16), W2aug=W2aug.astype(BF16),
        W3aug=W3aug, W4aug=W4aug, x_pad=x_pad,
        onesmat=onesmat, onesb16=np.ones((1, DSTW), BF16),
    )
    per_core = []
    for c in range(N_CORES):
        per_core.append(dict(
            idx16_1=l1_data[c]["idx16"], meta1=l1_data[c]["meta"],
            idx16_2=l2_data[c]["idx16"], meta2=l2_data[c]["meta"],
            pool_mask_bcast=np.tile(pool_masks[c][:, None, :], (1, 128, 1)),
            node_start=node_start[c], n_nodes=nodes_per_core[c],
        ))
    return cfg, per_core, shared


FP32 = mybir.dt.float32
FP32R = mybir.dt.float32r
BF16D = mybir.dt.bfloat16
I16 = mybir.dt.int16
AF = mybir.ActivationFunctionType
ALU = mybir.AluOpType


def build_kernel(cfg, n_cores=8):
    G = cfg["G_PER_CORE"]
    N, NMAX = cfg["N"], cfg["NMAX"]
    n_win1, n_win2 = cfg["n_win1"], cfg["n_win2"]
    T1, T2 = cfg["T1"], cfg["T2"]
    n_win2p = cfg["n_win2_pad16"]
    sched1, sched2 = cfg["sched1"], cfg["sched2"]
    piece_rows = cfg["piece_rows"]
    piece_w0 = cfg["piece_w0"]
    piece_r0 = cfg["piece_r0"]

    nc = bacc.Bacc("TRN2", target_bir_lowering=False, debug=False,
                   num_devices=n_cores, num_swdge_queues=4)

    # ---- I/O ----
    x_pad = nc.dram_tensor("x_pad", [N, F1P], BF16D, kind="ExternalInput")
    idx1 = nc.dram_tensor("idx1", [128, cfg["n_idx16_1"]], I16,
                          kind="ExternalInput")
    idx2 = nc.dram_tensor("idx2", [128, cfg["n_idx16_2"]], I16,
                          kind="ExternalInput")
    meta1 = nc.dram_tensor("meta1", [n_win1, 128, 4 * T1], FP32,
                           kind="ExternalInput")
    meta2 = nc.dram_tensor("meta2", [n_win2, 128, 4 * T2], FP32,
                           kind="ExternalInput")
    pmask = nc.dram_tensor("pmask", [G, 128, n_win2p], FP32,
                           kind="ExternalInput")
    w1aug = nc.dram_tensor("w1aug", [80, F2S], BF16D, kind="ExternalInput")
    w2aug = nc.dram_tensor("w2aug", [304, 384], BF16D, kind="ExternalInput")
    w3aug = nc.dram_tensor("w3aug", [304, 1024], FP32, kind="ExternalInput")
    w4aug = nc.dram_tensor("w4aug", [1152, 128], FP32, kind="ExternalInput")
    onesmat = nc.dram_tensor("onesmat", [128, DSTW], FP32,
                             kind="ExternalInput")
    onesb16 = nc.dram_tensor("onesb16", [1, DSTW], BF16D,
                             kind="ExternalInput")
    z_out = nc.dram_tensor("z", [G, 128], FP32, kind="ExternalOutput")

    with tile.TileContext(nc) as tc, \
         tc.tile_pool(name="dram", bufs=1, space="DRAM") as drp, \
         tc.tile_pool(name="consts", bufs=1) as consts:
        # ---- persistent DRAM intermediates ----
        h1_me = [drp.tile([piece_rows[q], F2S], BF16D, name=f"h1me{q}")
                 for q in range(NPIECE)]
        h1_full = [drp.tile([n_cores * piece_rows[q], F2S], BF16D,
                            addr_space="Shared", name=f"h1full{q}")
                   for q in range(NPIECE)]

        iota_i32 = consts.tile([128, DSTW], mybir.dt.int32)
        nc.gpsimd.iota(iota_i32[:], [[1, DSTW]], base=0, channel_multiplier=0)
        iota_sb = consts.tile([128, DSTW], BF16D)
        nc.vector.tensor_copy(iota_sb[:], iota_i32[:])

        w1_sb = consts.tile([80, F2S], BF16D)
        nc.sync.dma_start(w1_sb[:], w1aug[:])
        w2_sb = []
        for k in range(3):
            rows = [128, 128, 48][k]
            t = consts.tile([rows, 384], BF16D, name=f"w2_sb{k}")
            nc.sync.dma_start(t[:], w2aug[k * 128 : k * 128 + rows, :])
            w2_sb.append(t)
        w2b_sb = consts.tile([1, 384], BF16D)
        nc.sync.dma_start(w2b_sb[:], w2aug[300:301, :])
        w3_sb = []
        for k in range(3):
            rows = [128, 128, 48][k]
            t = consts.tile([rows, 1024], FP32R, name=f"w3_sb{k}")
            nc.sync.dma_start(t[:], w3aug[k * 128 : k * 128 + rows, :].bitcast(FP32R))
            w3_sb.append(t)
        w3b_sb = consts.tile([1, 1024], FP32R)
        nc.sync.dma_start(w3b_sb[:], w3aug[300:301, :].bitcast(FP32R))
        w4_sb = []
        for k in range(9):
            t = consts.tile([128, 128], FP32R, name=f"w4_sb{k}")
            nc.sync.dma_start(t[:], w4aug[k * 128 : (k + 1) * 128, :].bitcast(FP32R))
            w4_sb.append(t)
        ones256_sb = consts.tile([1, DSTW], BF16D)
        nc.sync.dma_start(ones256_sb[:], onesb16[0:1, :])
        ones_sb = consts.tile([128, G], FP32R)
        nc.sync.dma_start(ones_sb[:], onesmat[:, 0:G].bitcast(FP32R))

        pooled_win = [consts.tile([128, n_win2p], FP32, name=f"pw{m}")
                      for m in range(3)]
        for m in range(3):
            nc.vector.memset(pooled_win[m][:], -3.0e38)

        # =============== generic window gather ===============
        qctr = [0]

        def gather_window(pools, w, sched, idx_hbm, tables, T, F, tag):
            """tables: list of (table_ap, rows) per cell."""
            gpool, ipool = pools
            ent, tot = sched[w]
            gbuf = gpool.tile([128, T, F], BF16D, tag="gbuf",
                              name=f"gbuf_{tag}_{w}",
                              padded_shape=[128, T, F])
            c16_0 = ent[0][3]
            c16_n = ent[-1][3] + ent[-1][1] // 16
            itile = ipool.tile([128, c16_n - c16_0], I16, tag="idx",
                               name=f"idx_{tag}_{w}")
            nc.sync.dma_start(itile[:], idx_hbm[:, c16_0:c16_n])
            for (k, cap, slot, c16) in ent:
                src = tables[k]
                for off in range(0, cap, GMAXI):
                    sub = min(GMAXI, cap - off)
                    so = slot + off
                    co = c16 - c16_0 + off // 16
                    nc.gpsimd.dma_gather(
                        gbuf[:, so // 128 : (so + sub) // 128, :],
                        src,
                        itile[:, co : co + sub // 16],
                        sub, sub, F,
                        queue_num=qctr[0] % 4,
                    )
                    qctr[0] += 1
            return gbuf, tot // 128

        # =============== Phase 1: L1 aggregation + fused h1 ===============
        x_tables = []
        for k in range(cfg["n_chunks_x"]):
            lo = k * CHUNK
            hi = min(lo + CHUNK, N)
            x_tables.append(x_pad[lo:hi, :])
        with tc.tile_pool(name="gp1", bufs=5) as gpool, \
             tc.tile_pool(name="ip1", bufs=6) as ipool, \
             tc.tile_pool(name="mp1", bufs=4) as mpool, \
             tc.tile_pool(name="sp1", bufs=16) as spool, \
             tc.tile_pool(name="up1", bufs=8) as upool, \
             tc.tile_pool(name="ps_agg1", bufs=4, space="PSUM") as ps_agg, \
             tc.tile_pool(name="ps_h1", bufs=2, space="PSUM") as ps_h1, \
             tc.tile_pool(name="sb_ep1", bufs=3) as sb_ep:
            for w in range(n_win1):
                gbuf, nt = gather_window((gpool, ipool), w, sched1, idx1,
                                         x_tables, T1, F1P, "l1")
                meta = mpool.tile([128, 4 * T1], FP32, tag="meta",
                                  name=f"m1_{w}")
                nc.sync.dma_start(meta[:], meta1[w])
                agg = ps_agg.tile([128, DSTW], FP32, tag="agg1",
                                  name=f"agg1_{w}")
                for t in range(nt):
                    S = spool.tile([128, DSTW], BF16D, tag="S",
                                   name=f"S1_{w}_{t}")
                    if (w + t) % 2 == 0:
                        nc.vector.tensor_scalar(
                            S[:], iota_sb[:], meta[:, t : t + 1],
                            meta[:, T1 + t : T1 + t + 1], ALU.is_equal,
                            ALU.mult)
                    else:
                        # S = relu(norm - norm*(iota-dstl)^2) on scalar engine
                        u = upool.tile([128, DSTW], BF16D, tag="u",
                                       name=f"u1_{w}_{t}")
                        nc.scalar.activation(
                            u[:], iota_sb[:], AF.Square,
                            bias=meta[:, 2 * T1 + t : 2 * T1 + t + 1])
                        nc.scalar.activation(
                            S[:], u[:], AF.Relu,
                            bias=meta[:, T1 + t : T1 + t + 1],
                            scale=meta[:, 3 * T1 + t : 3 * T1 + t + 1])
                    nc.tensor.matmul(agg[:], gbuf[:, t, :], S[:],
                                     start=(t == 0), stop=(t == nt - 1))
                # epilogue: agg [80, 256] -> h1 blocks [128, 384] -> h1_me
                agg_sb = sb_ep.tile([80, DSTW], BF16D, tag="agg_sb",
                                    name=f"as1_{w}")
                nc.scalar.activation(agg_sb[:], agg[0:80, :], AF.Copy)
                q = int(np.searchsorted(piece_w0[1:], w, side="right"))
                for h in range(2):
                    hp = ps_h1.tile([128, F2S], FP32, tag="h1p",
                                    name=f"h1p_{w}_{h}")
                    nc.tensor.matmul(hp[:], agg_sb[:, h * 128 : (h + 1) * 128],
                                     w1_sb[:], start=True, stop=True)
                    h1b = sb_ep.tile([128, F2S], BF16D, tag="h1b",
                                     name=f"h1b_{w}_{h}")
                    nc.scalar.activation(h1b[:], hp[:], AF.Relu)
                    r0 = w * DSTW + h * 128 - piece_r0[q]
                    nc.sync.dma_start(h1_me[q][r0 : r0 + 128, :], h1b[:])

        # =============== Phase 2: piecewise AllGather ===============
        for q in range(NPIECE):
            nc.gpsimd.collective_compute(
                "AllGather", ALU.bypass,
                replica_groups=[list(range(n_cores))],
                ins=[h1_me[q].opt()],
                outs=[h1_full[q].opt()],
            )

        # =============== Phase 3: L2 aggregation + W2 + window pooling =====
        FCH = [(0, 128), (128, 256), (256, 384)]
        KCH = [(0, 128), (128, 256), (256, 300)]
        h1_tables = [h1_full[q][:, :] for q in range(NPIECE)]
        with tc.tile_pool(name="gp2", bufs=4) as gpool, \
             tc.tile_pool(name="ip2", bufs=6) as ipool, \
             tc.tile_pool(name="mp2", bufs=4) as mpool, \
             tc.tile_pool(name="sp2", bufs=16) as spool, \
             tc.tile_pool(name="up2", bufs=8) as upool, \
             tc.tile_pool(name="ps_agg2", bufs=2, space="PSUM") as ps_agg2, \
             tc.tile_pool(name="ps_h2", bufs=2, space="PSUM") as ps_h2, \
             tc.tile_pool(name="sb_ep2", bufs=2) as sb_ep2:
            for w in range(n_win2):
                gbuf, nt = gather_window((gpool, ipool), w, sched2, idx2,
                                         h1_tables, T2, F2S, "l2")
                meta = mpool.tile([128, 4 * T2], FP32, tag="meta",
                                  name=f"m2_{w}")
                nc.sync.dma_start(meta[:], meta2[w])
                aggs = []
                for fi, (f0, f1) in enumerate(FCH):
                    aggs.append(ps_agg2.tile([f1 - f0, DSTW], FP32,
                                             tag=f"agg2_{fi}",
                                             name=f"agg2_{w}_{fi}"))
                for t in range(nt):
                    S = spool.tile([128, DSTW], BF16D, tag="S",
                                   name=f"S2_{w}_{t}")
                    if (w + t) % 2 == 0:
                        nc.vector.tensor_scalar(
                            S[:], iota_sb[:], meta[:, t : t + 1],
                            meta[:, T2 + t : T2 + t + 1], ALU.is_equal,
                            ALU.mult)
                    else:
                        u = upool.tile([128, DSTW], BF16D, tag="u",
                                       name=f"u2_{w}_{t}")
                        nc.scalar.activation(
                            u[:], iota_sb[:], AF.Square,
                            bias=meta[:, 2 * T2 + t : 2 * T2 + t + 1])
                        nc.scalar.activation(
                            S[:], u[:], AF.Relu,
                            bias=meta[:, T2 + t : T2 + t + 1],
                            scale=meta[:, 3 * T2 + t : 3 * T2 + t + 1])
                    for fi, (f0, f1) in enumerate(FCH):
                        nc.tensor.matmul(aggs[fi][:], gbuf[:, t, f0:f1], S[:],
                                         start=(t == 0), stop=(t == nt - 1))
                a_sb = []
                for fi, (f0, f1) in enumerate(FCH):
                    t_ = sb_ep2.tile([f1 - f0, DSTW], BF16D, tag=f"a2sb_{fi}",
                                     name=f"a2sb_{w}_{fi}")
                    nc.scalar.activation(t_[:], aggs[fi][:], AF.Copy)
                    a_sb.append(t_)
                for m, (m0, m1) in enumerate([(0, 128), (128, 256),
                                              (256, 384)]):
                    hp = ps_h2.tile([m1 - m0, DSTW], FP32, tag="h2p",
                                    name=f"h2p_{w}_{m}")
                    for ki, (k0, k1) in enumerate(KCH):
                        nc.tensor.matmul(
                            hp[:], w2_sb[ki][0 : k1 - k0, m0:m1],
                            a_sb[ki][0 : k1 - k0, :],
                            start=(ki == 0), stop=False)
                    nc.tensor.matmul(hp[:], w2b_sb[:, m0:m1], ones256_sb[:],
                                     start=False, stop=True)
                    h2sb = sb_ep2.tile([m1 - m0, DSTW], FP32, tag="h2sb",
                                       name=f"h2sb_{w}_{m}")
                    nc.scalar.activation(h2sb[:], hp[:], AF.Relu)
                    nc.vector.tensor_reduce(
                        pooled_win[m][0 : m1 - m0, w : w + 1], h2sb[:],
                        axis=mybir.AxisListType.X, op=ALU.max)

        # =============== Phase 4: pool combine + MLP ===============
        with tc.tile_pool(name="pm", bufs=3) as pmp, \
             tc.tile_pool(name="pool5", bufs=2) as p5, \
             tc.tile_pool(name="ps_z", bufs=2, space="PSUM") as psz, \
             tc.tile_pool(name="zsb", bufs=2) as zsb:
            pooledT = [p5.tile([128, G], FP32, tag=f"pT{m}", bufs=1,
                               name=f"pooledT{m}") for m in range(3)]
            for g in range(G):
                msk = pmp.tile([128, n_win2p], FP32, tag="msk", name=f"msk_{g}")
                nc.sync.dma_start(msk[:], pmask[g])
                for m in range(3):
                    tmp = pmp.tile([128, n_win2p], FP32, tag="tmp",
                                   name=f"tmp_{g}_{m}")
                    nc.vector.tensor_tensor(tmp[:], pooled_win[m][:], msk[:],
                                            ALU.add)
                    nc.vector.tensor_reduce(
                        pooledT[m][:, g : g + 1], tmp[:],
                        axis=mybir.AxisListType.X, op=ALU.max)
            pooledTr = [p5.tile([128, G], FP32R, tag=f"pTr{m}", bufs=1,
                                name=f"pooledTr{m}") for m in range(3)]
            for m in range(3):
                nc.scalar.activation(pooledTr[m][:], pooledT[m][:], AF.Relu)
            z1t = []
            for mi in range(8):
                zp = psz.tile([128, G], FP32, tag="z1p", name=f"z1p_{mi}")
                for ki, (k0, k1) in enumerate(KCH):
                    nc.tensor.matmul(
                        zp[:], w3_sb[ki][0 : k1 - k0, mi * 128 : (mi + 1) * 128],
                        pooledTr[ki][0 : k1 - k0, :],
                        start=(ki == 0), stop=False)
                nc.tensor.matmul(zp[:], w3b_sb[:, mi * 128 : (mi + 1) * 128],
                                 ones_sb[0:1, :], start=False, stop=True)
                zt = zsb.tile([128, G], FP32R, tag=f"z1t{mi}", bufs=1,
                              name=f"z1t_{mi}")
                nc.scalar.activation(zt[:], zp[:], AF.Relu)
                z1t.append(zt)
            zp2 = psz.tile([G, 128], FP32, tag="z2p", name="z2p")
            for ki in range(9):
                lhsT = z1t[ki][:] if ki < 8 else ones_sb[:]
                nc.tensor.matmul(zp2[:], lhsT, w4_sb[ki][:],
                                 start=(ki == 0), stop=(ki == 8))
            zfin = zsb.tile([G, 128], FP32, tag="zfin", name="zfin")
            nc.scalar.activation(zfin[:], zp2[:], AF.Relu)
            nc.sync.dma_start(z_out[:], zfin[:])

    nc.compile()
    nc.generate_event_semaphores()
    return nc


# ======================= public entry point =======================
_NC_CACHE = {}


def _make_in_maps(cfg, per_core, shared):
    base = dict(
        x_pad=shared["x_pad"], w1aug=shared["W1aug"], w2aug=shared["W2aug"],
        w3aug=shared["W3aug"], w4aug=shared["W4aug"],
        onesmat=shared["onesmat"], onesb16=shared["onesb16"],
    )
    maps = []
    for pc in per_core:
        m = dict(base)
        m["idx1"] = pc["idx16_1"]
        m["idx2"] = pc["idx16_2"]
        m["meta1"] = pc["meta1"]
        m["meta2"] = pc["meta2"]
        m["pmask"] = pc["pool_mask_bcast"]
        maps.append(m)
    return maps


def kernel(x, edge_index, batch, W1, b1, W2, b2, W3, b3, W4, b4,
           trace=False):
    weights = dict(W1=np.asarray(W1, np.float32), b1=np.asarray(b1, np.float32),
                   W2=np.asarray(W2, np.float32), b2=np.asarray(b2, np.float32),
                   W3=np.asarray(W3, np.float32), b3=np.asarray(b3, np.float32),
                   W4=np.asarray(W4, np.float32), b4=np.asarray(b4, np.float32))
    n_graphs = 512
    n_cores = 8
    cfg, per_core, shared = build_plan(
        np.asarray(x, np.float32), np.asarray(edge_index), np.asarray(batch),
        weights, n_graphs=n_graphs, n_cores=n_cores)
    key = (cfg["N"], cfg["NMAX"], cfg["n_win1"], cfg["n_win2"], cfg["T1"],
           cfg["T2"], cfg["n_idx16_1"], cfg["n_idx16_2"], cfg["n_win2_pad16"])
    if key not in _NC_CACHE:
        _NC_CACHE[key] = build_kernel(cfg, n_cores=n_cores)
    nc = _NC_CACHE[key]
    maps = _make_in_maps(cfg, per_core, shared)
    res = run_bass_kernel_spmd(nc, maps, core_ids=list(range(n_cores)),
                               trace=trace)
    z = np.concatenate([res.results[c]["z"] for c in range(n_cores)], axis=0)
    if trace:
        kernel.last_results = res
    return z.astype(np.float32)
